# revision 26
# baseline (speedup 1.0000x reference)
"""Complex LSTM cell (CLSTMCell) Trainium2 kernel — fp8 DoubleRow edition.

Full inputs in, full outputs out. Data-parallel over batch: B=4096 rows
sharded 512/core across 8 NeuronCores; weights replicated (host pre-packed
into a matmul-friendly fp8 layout).

Math: with X1=[xr|hr], X2=[xi|hi] ([B,2048]) and W1=[Ur;Wr], W2=[Ui;Wi]
([2048,4096]), the complex gate projection is computed via Karatsuba:
  P1 = X1@W1, P2 = X2@W2, P3 = (X1+X2)@(W1+W2)
  Zr = P1 - P2 (+ br),  Zi = P3 - P1 - P2 (+ bi)
i.e. 3 real matmuls instead of 4 (25% FLOP cut).

Matmuls run in fp8-e4m3 with MatmulPerfMode.DoubleRow (two k-subtiles per
instruction at 0.5 cycles/row = 4x bf16 PE throughput). fp8's 3-bit
mantissa alone is too coarse, so per (mat, gate) a correction class is
assigned (CFG below, solved by an offline LP against the measured output
sensitivity of each gate; h_t rel err is the binding constraint):

  P (plain):     main pass Xh@Wh only (Xh = fp8(4X), Wh = fp8(1024W)).
  S (split-main): main pass uses Wt = fp8(alpha*1024W) (a FRESH fp8
     quantization, so alpha is unconstrained — unlike the usual trick of
     halving Wh, which pins alpha to 1/2) plus ONE correction pass A@B:
       A = fp8(Xh + Xl/t),  B = fp8(t*1024W - e'),  e' = Wt - alpha*1024W
     with t = 1-alpha. Expanding, main+corr = X@W + noise where the e'
     terms cancel exactly (e' is known offline) and the residual is
     ~[2t^2 + (delta*alpha/t)^2]*delta^2, minimized at t ~ sqrt(delta):
     t=1/8 measures 0.035x the plain-class residual err^2 — 7x better
     than the classic half-Wh mixed pass — for the same single extra
     pass. This obsoletes the 2-pass exact correction everywhere.

Partial-k coverage (kcov, in DoubleRow k-pairs of 256) interpolates cost
vs err^2 linearly: covered k-tiles use Wt + A@B, uncovered use plain Wh.

Weight columns are interleaved as c = oblk*512 + gate*128 + (o % 128)
with gate order [i,f,o,a]. Per-mat slab layout: [main(512) | B cols of
full-k S ranges]; the partial-k f-gate B block ships as a tiny separate
tensor (bf0) so the slab stays rectangular without dead bytes.
"""

import sys

for _p in ("/opt/trn_rl_repo",):
    if _p not in sys.path:
        sys.path.insert(0, _p)

import numpy as np
import ml_dtypes

import concourse.bass as bass
import concourse.mybir as mybir
from concourse.bass_utils import run_bass_kernel_spmd
from concourse.tile import TileContext

F32 = mybir.dt.float32
BF16 = mybir.dt.bfloat16
F8 = mybir.dt.float8e4
AFT = mybir.ActivationFunctionType
DR = mybir.MatmulPerfMode.DoubleRow

B = 4096
IN = 1024
H = 1024
G = 4
NCORES = 8
BL = B // NCORES          # 512 batch rows per core
MT = BL // 128            # 4 m-tiles per core
K = 2 * IN                # 2048 contraction dim (x|h concat)
KT = K // 128             # 16 k-tiles
OB = H // 128             # 8 o-blocks
NW = G * 128              # 512 matmul N (all gates for one o-block)
SX = 4.0
SW = 1024.0
SP = SX * SW
TS = 0.125                # M* split t; alpha = 1 - t
AL = 1.0 - TS

# (c0, c1, class, kcov) over the 512 gate columns, order [i, f, o, a].
# Offline-LP solution: sim h_t rel 1.942e-2 / c_t 1.788e-2 vs the 2e-2
# gate (deterministic inputs; hw-sim gap measured ~1e-7 in err^2).
CFG = [
    [(0, 128, "P", 0), (128, 256, "S", 1), (256, 512, "S", 8)],  # P1
    [(0, 256, "P", 0), (256, 512, "S", 8)],                      # P2
    [(0, 512, "S", 8)],                                          # P3
]


def _mat_passes(mat):
    """[(c0, c1, corr_off, kcov)] for slab-resident S ranges (kcov==8),
    plus slab width. Partial-k S ranges ride the bf0 side tensor."""
    off = NW
    passes = []
    for (c0, c1, cls, kcov) in CFG[mat]:
        if cls == "S" and kcov == 8:
            passes.append((c0, c1, off, kcov))
            off += c1 - c0
    return passes, off


MAT_PASSES = [_mat_passes(m) for m in range(3)]
WIDTHS = [MAT_PASSES[m][1] for m in range(3)]  # 768, 768, 1024
# the one partial-k S range (mat0 f-gate, kcov=1): B block [OB,128,2,128]
BF0_MAT, BF0_C0, BF0_C1, BF0_KCOV = 0, 128, 256, 1


def _split_multiwait_json(raw: bytes) -> bytes:
    """The walrus build in this container accepts at most one sem wait
    per instruction; Tile's scheduler packs several. Split the extras
    into preceding wait-only EventSemaphore instructions on the same
    engine (same semantics: the sequencer blocks on each in order)."""
    import orjson

    m = orjson.loads(raw)
    ctr = 0
    for fn in m["functions"]:
        for bb in fn["blocks"]:
            out = []
            for ins in bb["instructions"]:
                si = ins.get("sync_info")
                waits = si.get("on_wait") if si else None
                if waits and len(waits) > 1:
                    for w in waits[:-1]:
                        ctr += 1
                        nop = {
                            "engine": ins["engine"],
                            "ins": [],
                            "outs": [],
                            "name": f"{ins['name']}_sw{ctr}",
                            "opcode": "EventSemaphore",
                            "sync_info": {"on_update": [], "on_wait": [w]},
                        }
                        if "debug" in ins:
                            nop["debug"] = ins["debug"]
                        out.append(nop)
                    si["on_wait"] = [waits[-1]]
                out.append(ins)
            bb["instructions"] = out
    return orjson.dumps(m)


def _build_program():
    nc = bass.Bass()

    xh = [nc.dram_tensor(f"x{m}h", [KT, 128, BL], F8, kind="ExternalInput")
          for m in range(3)]
    xa = [nc.dram_tensor(f"x{m}a", [KT, 128, BL], F8, kind="ExternalInput")
          for m in range(3)]
    cx = nc.dram_tensor("cx", [BL, 2 * H], BF16, kind="ExternalInput")
    wt = [nc.dram_tensor(f"w{m}", [OB, 128, KT, WIDTHS[m]], F8,
                         kind="ExternalInput") for m in range(3)]
    bf0 = nc.dram_tensor("bf0", [OB, 128, 2 * BF0_KCOV, BF0_C1 - BF0_C0], F8,
                         kind="ExternalInput")
    bbc = nc.dram_tensor("bbc", [2, 128, G * H], BF16, kind="ExternalInput")
    h_out = nc.dram_tensor("h_out", [BL, 2 * H], BF16, kind="ExternalOutput")
    c_out = nc.dram_tensor("c_out", [BL, 2 * H], BF16, kind="ExternalOutput")

    with TileContext(nc) as tc:
        with (
            tc.tile_pool(name="const", bufs=2) as constp,
            tc.tile_pool(name="cres", bufs=2) as cresp,
            tc.tile_pool(name="xt", bufs=1) as xtp,
            tc.tile_pool(name="w0p", bufs=2) as w0p,
            tc.tile_pool(name="w1p", bufs=2) as w1p,
            tc.tile_pool(name="w2p", bufs=2) as w2p,
            tc.tile_pool(name="ep", bufs=2) as epp,
            tc.tile_pool(name="prod", bufs=2) as prodp,
            tc.tile_pool(name="ps_mm", bufs=6, space="PSUM") as psmm,
        ):
            wpools = [w0p, w1p, w2p]
            obres = {}
            bfts = {}

            def load_bf0(ob, eng):
                bft = constp.tile([128, 2 * BF0_KCOV, BF0_C1 - BF0_C0], F8,
                                  tag="bf0", name="bf0")
                eng.dma_start(out=bft[:], in_=bf0[ob])
                bfts[ob] = bft

            def load_ob_resources(ob):
                brt = constp.tile([128, NW], BF16, tag="bias_r", name="bias_r")
                bit = constp.tile([128, NW], BF16, tag="bias_i", name="bias_i")
                obw = slice(ob * NW, (ob + 1) * NW)
                nc.scalar.dma_start(out=brt[:], in_=bbc[0][:, obw])
                nc.scalar.dma_start(out=bit[:], in_=bbc[1][:, obw])
                if ob not in bfts:
                    load_bf0(ob, nc.scalar)
                cts = []
                for m in range(MT):
                    t = cresp.tile([128, 2, 128], BF16, tag=f"c_m{m}",
                                   name=f"c_m{m}")
                    src = cx[m * 128 : (m + 1) * 128, :].rearrange(
                        "r (two h) -> r two h", two=2
                    )[:, :, ob * 128 : (ob + 1) * 128]
                    nc.scalar.dma_start(out=t[:], in_=src)
                    cts.append(t)
                obres[ob] = (brt, bit, cts)

            xht = [xtp.tile([128, KT, BL], F8, tag=f"xh{m}", name=f"xh{m}")
                   for m in range(3)]
            att = [xtp.tile([128, KT, BL], F8, tag=f"xa{m}", name=f"xa{m}")
                   for m in range(3)]

            def emit_xload(src, dst, eng, quarters=4, peel=False, kt=KT):
                kq = kt // quarters
                for q in range(quarters):
                    if peel and q == 0:
                        eng.dma_start(
                            out=dst[:, 0:2, :],
                            in_=src[0:2].rearrange("kt p b -> p kt b"))
                        eng.dma_start(
                            out=dst[:, 2:4, :],
                            in_=src[2:4].rearrange("kt p b -> p kt b"))
                        continue
                    eng.dma_start(
                        out=dst[:, q * kq : (q + 1) * kq, :],
                        in_=src[q * kq : (q + 1) * kq].rearrange(
                            "kt p b -> p kt b"))

            def load_slab(ob, mat, chunks=1, eng2=None):
                """eng2: second DMA ring carrying the back half (catch-up
                relief for the SP weight stream)."""
                w = wpools[mat].tile([128, KT, WIDTHS[mat]], F8, tag="wslab",
                                     name=f"w{mat}slab")
                wsrc = wt[mat][ob].rearrange("p kt c -> p (kt c)")
                wdst = w.rearrange("p kt c -> p (kt c)")
                tot = KT * WIDTHS[mat]
                if eng2 is not None:
                    half = tot // 2
                    nc.sync.dma_start(out=wdst[:, :half], in_=wsrc[:, :half])
                    eng2.dma_start(out=wdst[:, half:], in_=wsrc[:, half:])
                    return w
                cs = tot // chunks
                for q in range(chunks):
                    nc.sync.dma_start(out=wdst[:, q * cs : (q + 1) * cs],
                                      in_=wsrc[:, q * cs : (q + 1) * cs])
                return w

            p1ps = [None] * MT   # P1 PSUM handles per m
            pa = [None] * MT     # phase-A products per m

            def emit_passes(ps, rows, mat, w, ob, which, start, stop,
                            c0lim=0, c1lim=NW):
                """which: subset of ('m','a','b'). b = the bf0 partial-k
                pass (mat0 f-gate). start/stop on first/last instruction."""
                plist = []
                if "m" in which:
                    plist.append(("m", c0lim, c1lim, 0, KT // 2))
                if "a" in which:
                    for (c0, c1, off, kcov) in MAT_PASSES[mat][0]:
                        d0, d1 = max(c0, c0lim), min(c1, c1lim)
                        if d0 < d1:
                            plist.append(("a", d0, d1, off + d0 - c0, kcov))
                if "b" in which and mat == BF0_MAT:
                    d0, d1 = max(BF0_C0, c0lim), min(BF0_C1, c1lim)
                    if d0 < d1:
                        plist.append(("b", d0, d1, d0 - BF0_C0, BF0_KCOV))
                for ci, (side, d0, d1, off, kcov) in enumerate(plist):
                    for kp in range(kcov):
                        k = 2 * kp
                        if side == "m":
                            lhsT = xht[mat][:, k : k + 2, rows]
                            rhs = w[:, k : k + 2, d0:d1]
                        elif side == "a":
                            lhsT = att[mat][:, k : k + 2, rows]
                            rhs = w[:, k : k + 2, off : off + d1 - d0]
                        else:
                            lhsT = att[mat][:, k : k + 2, rows]
                            rhs = bfts[ob][:, k : k + 2, off : off + d1 - d0]
                        nc.tensor.matmul(
                            ps[:, d0:d1],
                            lhsT=lhsT,
                            rhs=rhs,
                            start=(start and ci == 0 and kp == 0),
                            stop=(stop and ci == len(plist) - 1
                                  and kp == kcov - 1),
                            perf_mode=DR,
                        )

            def phase_a(ob, m, ps2):
                # everything that only needs P1/P2 (not P3); overlaps the
                # P3 matmuls. Gate cols: i=[0:128] f=[128:256] o=[256:384]
                # a=[384:512]. PSUM holds SP*z. Tensor ops may read only
                # ONE PSUM operand, so P1 gets an SBUF copy (DVE) first.
                brt, bit, cts = obres[ob]
                p1 = p1ps[m]
                zr = epp.tile([128, NW], F32, tag="zra", name="zra")
                nc.vector.tensor_sub(zr[:], p1[:], ps2[:])
                nc.gpsimd.tensor_add(zr[:], zr[:], brt[:])
                gr = epp.tile([128, NW], F32, tag=f"gr_{m}",
                              name=f"gr_{m}", bufs=1)
                nc.scalar.activation(gr[:, 0:384], zr[:, 0:384],
                                     AFT.Sigmoid, scale=1.0 / SP)
                nc.scalar.activation(gr[:, 384:512], zr[:, 384:512],
                                     AFT.Tanh, scale=1.0 / SP)
                q = epp.tile([128, NW], F32, tag=f"q_{m}",
                             name=f"q_{m}", bufs=1)
                nc.vector.tensor_add(q[:], p1[:], ps2[:])
                nc.gpsimd.tensor_sub(q[:], q[:], bit[:])
                cr = cts[m][:, 0, :]
                ci = cts[m][:, 1, :]
                ir_ = gr[:, 0:128]
                fr = gr[:, 128:256]
                ar = gr[:, 384:512]
                u1 = prodp.tile([128, 128], F32, tag=f"u1_{m}",
                                name=f"u1_{m}", bufs=1)
                u4 = prodp.tile([128, 128], F32, tag=f"u4_{m}",
                                name=f"u4_{m}", bufs=1)
                v1 = prodp.tile([128, 128], F32, tag=f"v1_{m}",
                                name=f"v1_{m}", bufs=1)
                nc.gpsimd.tensor_mul(u1[:], cr, fr)
                nc.gpsimd.tensor_mul(u4[:], ci, fr)
                nc.gpsimd.tensor_mul(v1[:], ar, ir_)
                pa[m] = (gr, q, u1, u4, v1)

            def phase_b(ob, m, rows, zi_parts, last_ob=False, tail=False):
                """zi_parts: [(ps, c0, c1, func)] in ACT-queue order. For
                tail=True the c-product chain rides Pool exclusively (DVE
                keeps only the PSUM subs) and is emitted after the o-gate
                part so the o sigmoid is not queued behind the ct tanhs."""
                gr, q, u1, u4, v1 = pa[m]
                zi = epp.tile([128, NW], F32, tag="zi", name="zi")
                gi = epp.tile([128, NW], F32, tag="gi", name="gi")
                D, P = nc.vector, nc.gpsimd
                # engine map: c-path ops
                if tail:
                    e_u3 = e_v3 = e_cfi = P
                else:
                    e_u3 = e_v3 = e_cfi = D
                emitted = []
                cpath = {}

                def c_products():
                    cr = obres[ob][2][m][:, 0, :]
                    ci = obres[ob][2][m][:, 1, :]
                    ii_ = gi[:, 0:128]
                    fi = gi[:, 128:256]
                    ai = gi[:, 384:512]
                    ir_ = gr[:, 0:128]
                    ar = gr[:, 384:512]
                    u2 = prodp.tile([128, 128], F32, tag="u2", name="u2", bufs=1)
                    u3 = prodp.tile([128, 128], F32, tag="u3", name="u3", bufs=1)
                    v2 = prodp.tile([128, 128], F32, tag="v2", name="v2", bufs=1)
                    v3 = prodp.tile([128, 128], F32, tag="v3", name="v3", bufs=1)
                    v4 = prodp.tile([128, 128], F32, tag="v4", name="v4", bufs=1)
                    P.tensor_mul(u2[:], ci, fi)
                    e_u3.tensor_mul(u3[:], cr, fi)
                    P.tensor_mul(v2[:], ai, ii_)
                    e_v3.tensor_mul(v3[:], ar, ii_)
                    P.tensor_mul(v4[:], ai, ir_)
                    cfr = prodp.tile([128, 128], F32, tag="cfr", name="cfr", bufs=1)
                    cfi = prodp.tile([128, 128], F32, tag="cfi", name="cfi", bufs=1)
                    air = prodp.tile([128, 128], F32, tag="air", name="air", bufs=1)
                    aii = prodp.tile([128, 128], F32, tag="aii", name="aii", bufs=1)
                    P.tensor_sub(cfr[:], u1[:], u2[:])
                    e_cfi.tensor_add(cfi[:], u3[:], u4[:])
                    P.tensor_sub(air[:], v1[:], v2[:])
                    P.tensor_add(aii[:], v3[:], v4[:])
                    ct2 = prodp.tile([128, 2, 128], BF16, tag="ct2", name="ct2")
                    ctr = ct2[:, 0, :]
                    cti = ct2[:, 1, :]
                    P.tensor_add(ctr, cfr[:], air[:])
                    P.tensor_add(cti, cfi[:], aii[:])
                    # one 256-wide tanh covers both halves of ct
                    tt = prodp.tile([128, 2, 128], F32, tag="tt", name="tt")
                    nc.scalar.activation(tt[:], ct2[:], AFT.Tanh)
                    cpath.update(ct2=ct2, tr=tt[:, 0, :], ti=tt[:, 1, :])

                for (psx, c0, c1, func) in zi_parts:
                    nc.vector.tensor_sub(zi[:, c0:c1], psx[:, c0:c1],
                                         q[:, c0:c1])
                    nc.scalar.activation(gi[:, c0:c1], zi[:, c0:c1],
                                         func, scale=1.0 / SP)
                    emitted.append((c0, c1))
                    have_a = any(cc1 >= 512 for (cc0, cc1) in emitted)
                    have_if = any(cc0 == 0 for (cc0, cc1) in emitted)
                    if have_a and have_if and not cpath and not tail:
                        c_products()
                if not cpath:
                    c_products()

                ct2, tr, ti = cpath["ct2"], cpath["tr"], cpath["ti"]
                orr = gr[:, 256:384]
                oi = gi[:, 256:384]
                ht2 = prodp.tile([128, 2, 128], BF16, tag="ht2", name="ht2")
                htr = ht2[:, 0, :]
                hti = ht2[:, 1, :]
                w1 = prodp.tile([128, 128], F32, tag="w1", name="w1")
                w2 = prodp.tile([128, 128], F32, tag="w2", name="w2")
                w3 = prodp.tile([128, 128], F32, tag="w3", name="w3")
                w4 = prodp.tile([128, 128], F32, tag="w4", name="w4")
                if tail:
                    P.tensor_mul(w1[:], orr, tr[:])
                    P.tensor_mul(w2[:], oi, ti[:])
                    P.tensor_sub(htr, w1[:], w2[:])
                    D.tensor_mul(w3[:], orr, ti[:])
                    D.tensor_mul(w4[:], oi, tr[:])
                    D.tensor_add(hti, w3[:], w4[:])
                else:
                    D.tensor_mul(w1[:], orr, tr[:])
                    P.tensor_mul(w2[:], oi, ti[:])
                    D.tensor_sub(htr, w1[:], w2[:])
                    P.tensor_mul(w3[:], orr, ti[:])
                    D.tensor_mul(w4[:], oi, tr[:])
                    P.tensor_add(hti, w3[:], w4[:])
                if last_ob:
                    oe1, oe2 = nc.sync, nc.sync
                else:
                    oe1, oe2 = nc.scalar, nc.gpsimd
                hdst = h_out[rows, :].rearrange(
                    "r (two h) -> r two h", two=2
                )[:, :, ob * 128 : (ob + 1) * 128]
                cdst = c_out[rows, :].rearrange(
                    "r (two h) -> r two h", two=2
                )[:, :, ob * 128 : (ob + 1) * 128]
                oe1.dma_start(out=hdst, in_=ht2[:])
                oe2.dma_start(out=cdst, in_=ct2[:])

            def finish_mat(ob, mat, m, ps, last_ob=False):
                if mat == 0:
                    p1c = epp.tile([128, NW], F32, tag=f"p1_{m}",
                                   name=f"p1_{m}", bufs=1)
                    nc.vector.tensor_copy(p1c[:], ps[:])
                    p1ps[m] = p1c
                elif mat == 1:
                    phase_a(ob, m, ps)
                else:
                    phase_b(ob, m, slice(m * 128, (m + 1) * 128),
                            [(ps, 384, 512, AFT.Tanh),
                             (ps, 0, 384, AFT.Sigmoid)],
                            last_ob=last_ob)

            # ---------------- emission ----------------
            emit_xload(xh[0], xht[0], nc.scalar, peel=True)
            emit_xload(xa[0], att[0], nc.gpsimd)
            load_bf0(0, nc.gpsimd)

            # ob0 mat0: mains deferred from corr passes (the A/B operands
            # land after the first mains start). Only mat0 defers — with
            # P1 banks held to phase A, deferring mat1 would deadlock the
            # in-order PE queue on PSUM-buffer reuse.
            w00 = load_slab(0, 0, chunks=16)
            ob0_ps = []
            for m in range(MT):
                ps = psmm.tile([128, NW], F32, tag="mm", name="mm")
                emit_passes(ps, slice(m * 128, (m + 1) * 128), 0, w00, 0,
                            ("m",), start=True, stop=False)
                ob0_ps.append(ps)
            for m in range(MT):
                emit_passes(ob0_ps[m], slice(m * 128, (m + 1) * 128), 0,
                            w00, 0, ("a", "b"), start=False, stop=True)
                finish_mat(0, 0, m, ob0_ps[m])
            # xh1 halves ride both rings so mat1's mains unblock sooner;
            # w1(ob0)'s back half rides the ACT ring (ahead of xa1/bias in
            # that queue) so the SP stream reaches w2(ob0) 2.4us earlier
            emit_xload(xh[1][:8], xht[1][:, :8, :], nc.scalar, quarters=2,
                       kt=8)
            emit_xload(xh[1][8:], xht[1][:, 8:, :], nc.gpsimd, quarters=2,
                       kt=8)
            w1_0 = load_slab(0, 1, eng2=nc.scalar)
            emit_xload(xa[1], att[1], nc.scalar)
            emit_xload(xh[2], xht[2], nc.gpsimd)
            emit_xload(xa[2], att[2], nc.gpsimd)
            load_ob_resources(0)
            for mat, w in ((1, w1_0), (2, load_slab(0, 2, chunks=4))):
                for m in range(MT):
                    ps = psmm.tile([128, NW], F32, tag="mm", name="mm")
                    emit_passes(ps, slice(m * 128, (m + 1) * 128), mat, w, 0,
                                ("m", "a", "b"), start=True, stop=True)
                    finish_mat(0, mat, m, ps)

            for ob in range(1, OB):
                last = ob == OB - 1
                for mat in range(3):
                    # the SP weight stream runs behind after ob0's bursty
                    # start; the big mat2 slabs of ob1-2 ride two rings
                    eng2 = nc.gpsimd if (mat == 2 and ob <= 2) else None
                    w = load_slab(ob, mat, eng2=eng2)
                    if mat == 0:
                        load_ob_resources(ob)
                    for m in range(MT):
                        rows = slice(m * 128, (m + 1) * 128)
                        if last and mat == 2 and m >= 2:
                            # tail: three accumulation groups; the h-only
                            # o-gate closes last so the c-path overlaps
                            # the o matmuls
                            psIF = psmm.tile([128, NW], F32, tag="mm", name="mm")
                            emit_passes(psIF, rows, mat, w, ob, ("m", "a", "b"),
                                        True, True, c0lim=0, c1lim=256)
                            psA = psmm.tile([128, NW], F32, tag="mm", name="mm")
                            emit_passes(psA, rows, mat, w, ob, ("m", "a", "b"),
                                        True, True, c0lim=384, c1lim=512)
                            psO = psmm.tile([128, NW], F32, tag="mm", name="mm")
                            emit_passes(psO, rows, mat, w, ob, ("m", "a", "b"),
                                        True, True, c0lim=256, c1lim=384)
                            phase_b(ob, m, rows,
                                    [(psIF, 0, 256, AFT.Sigmoid),
                                     (psA, 384, 512, AFT.Tanh),
                                     (psO, 256, 384, AFT.Sigmoid)],
                                    last_ob=True, tail=True)
                        else:
                            ps = psmm.tile([128, NW], F32, tag="mm", name="mm")
                            emit_passes(ps, rows, mat, w, ob, ("m", "a", "b"),
                                        True, True)
                            finish_mat(ob, mat, m, ps, last_ob=last)
    return nc


_NC_CACHE = None


def _get_program():
    global _NC_CACHE
    if _NC_CACHE is None:
        nc = _build_program()
        fixed = _split_multiwait_json(nc.to_json_bytes())
        nc.to_json_bytes = lambda: fixed
        _NC_CACHE = nc
    return _NC_CACHE


F8NP = ml_dtypes.float8_e4m3


def _split8(a):
    a = a * SX
    ah = a.astype(F8NP)
    ahf = ah.astype(np.float32)
    al = (a - ahf).astype(F8NP)
    aa = (ahf + al.astype(np.float32) / TS).astype(F8NP)
    return ah, aa


def _pack_weights(Uw_r, Uw_i, Ub_r, Ub_i, Ww_r, Ww_i, Wb_r, Wb_i):
    GORD = [1, 0, 3, 2]  # column blocks [i, f, o, a]

    def interleave_cols(Wg):  # [2048, G, H] -> [2048, GH]
        return (
            Wg.reshape(K, G, OB, 128)[:, GORD]
            .transpose(0, 2, 1, 3)
            .reshape(K, G * H)
        )

    Wr = np.concatenate(
        [np.transpose(Uw_r, (2, 0, 1)), np.transpose(Ww_r, (2, 0, 1))], axis=0
    )
    Wi = np.concatenate(
        [np.transpose(Uw_i, (2, 0, 1)), np.transpose(Ww_i, (2, 0, 1))], axis=0
    )
    W1 = interleave_cols(Wr) * SW
    W2 = interleave_cols(Wi) * SW
    W3 = W1 + W2
    Wall = [W1, W2, W3]

    def slabify(Wm):  # [K, G*H] -> [OB, 128, KT, NW]
        return Wm.reshape(KT, 128, OB, NW).transpose(2, 1, 0, 3)

    slabs = []
    bf0_pack = None
    for m in range(3):
        W = Wall[m]
        Wh8 = W.astype(F8NP)
        Wt8 = (AL * W).astype(F8NP)
        ep = Wt8.astype(np.float32) - AL * W
        B8 = (TS * W - ep).astype(F8NP)
        # main region: per range, per k: Wt for covered k, Wh beyond
        main = Wh8.astype(np.float32).reshape(K, OB, NW)
        for (c0, c1, cls, kcov) in CFG[m]:
            if cls == "S":
                main[: kcov * 256, :, c0:c1] = (
                    Wt8.astype(np.float32).reshape(K, OB, NW)[: kcov * 256, :, c0:c1]
                )
        parts = [slabify(main.reshape(K, G * H).astype(F8NP))]
        for (c0, c1, off, kcov) in MAT_PASSES[m][0]:
            parts.append(slabify(B8)[:, :, :, c0:c1])
        slabs.append(np.ascontiguousarray(np.concatenate(parts, axis=-1)))
        if m == BF0_MAT:
            # [OB, 128, 2*kcov, cols]
            bf0_pack = np.ascontiguousarray(
                slabify(B8)[:, :, : 2 * BF0_KCOV, BF0_C0:BF0_C1]
            )

    def interleave_bias(b):  # [G, H] -> [GH] interleaved, pre-scaled
        return b.reshape(G, OB, 128)[GORD].transpose(1, 0, 2).reshape(G * H)

    br = interleave_bias((Ub_r + Wb_r) * SP)
    bi = interleave_bias((Ub_i + Wb_i) * SP)
    bbc = np.ascontiguousarray(np.broadcast_to(
        np.stack([br, bi])[:, None, :], (2, 128, G * H)
    ).astype(ml_dtypes.bfloat16))
    return slabs, bf0_pack, bbc


def kernel(input, h_x, c_x, Uw_r, Uw_i, Ub_r, Ub_i, Ww_r, Ww_i, Wb_r, Wb_i,
           _trace=False):
    input = np.asarray(input, dtype=np.float32)
    h_x = np.asarray(h_x, dtype=np.float32)
    c_x = np.asarray(c_x, dtype=np.float32)
    slabs, bf0_pack, bpk = _pack_weights(
        np.asarray(Uw_r, np.float32), np.asarray(Uw_i, np.float32),
        np.asarray(Ub_r, np.float32), np.asarray(Ub_i, np.float32),
        np.asarray(Ww_r, np.float32), np.asarray(Ww_i, np.float32),
        np.asarray(Wb_r, np.float32), np.asarray(Wb_i, np.float32),
    )

    X1 = np.concatenate([input[:, :IN], h_x[:, :H]], axis=1)
    X2 = np.concatenate([input[:, IN:], h_x[:, H:]], axis=1)
    X3 = X1 + X2
    xparts = [_split8(X) for X in (X1, X2, X3)]

    in_maps = []
    for c in range(NCORES):
        rows = slice(c * BL, (c + 1) * BL)
        im = {
            "cx": np.ascontiguousarray(c_x[rows].astype(ml_dtypes.bfloat16)),
            "bbc": bpk,
            "bf0": bf0_pack,
        }
        for m in range(3):
            im[f"w{m}"] = slabs[m]
            xhm, xam = xparts[m]
            im[f"x{m}h"] = np.ascontiguousarray(
                xhm[rows].T.reshape(KT, 128, BL)
            )
            im[f"x{m}a"] = np.ascontiguousarray(
                xam[rows].T.reshape(KT, 128, BL)
            )
        in_maps.append(im)

    nc = _get_program()
    res = run_bass_kernel_spmd(
        nc, in_maps, core_ids=list(range(NCORES)), trace=_trace
    )
    h_t = np.concatenate(
        [res.results[i]["h_out"].astype(np.float32) for i in range(NCORES)],
        axis=0,
    )
    c_t = np.concatenate(
        [res.results[i]["c_out"].astype(np.float32) for i in range(NCORES)],
        axis=0,
    )
    if _trace:
        kernel._last_results = res
    return h_t, c_t


# revision 27
# speedup vs baseline: 1.0284x; 1.0284x over previous
"""Complex LSTM cell (CLSTMCell) Trainium2 kernel — fp8 DoubleRow edition.

Full inputs in, full outputs out. Data-parallel over batch: B=4096 rows
sharded 512/core across 8 NeuronCores; weights replicated (host pre-packed
into a matmul-friendly fp8 layout).

Math: with X1=[xr|hr], X2=[xi|hi] ([B,2048]) and W1=[Ur;Wr], W2=[Ui;Wi]
([2048,4096]), the complex gate projection is computed via Karatsuba:
  P1 = X1@W1, P2 = X2@W2, P3 = (X1+X2)@(W1+W2)
  Zr = P1 - P2 (+ br),  Zi = P3 - P1 - P2 (+ bi)
i.e. 3 real matmuls instead of 4 (25% FLOP cut).

Matmuls run in fp8-e4m3 with MatmulPerfMode.DoubleRow (two k-subtiles per
instruction at 0.5 cycles/row = 4x bf16 PE throughput). fp8's 3-bit
mantissa alone is too coarse, so per (mat, gate) a correction class is
assigned (CFG below, solved by an offline LP against the measured output
sensitivity of each gate; h_t rel err is the binding constraint):

  P (plain):     main pass Xh@Wh only (Xh = fp8(4X), Wh = fp8(1024W)).
  S (split-main): main pass uses Wt = fp8(alpha*1024W) (a FRESH fp8
     quantization, so alpha is unconstrained — unlike the usual trick of
     halving Wh, which pins alpha to 1/2) plus ONE correction pass A@B:
       A = fp8(Xh + Xl/t),  B = fp8(t*1024W - e'),  e' = Wt - alpha*1024W
     with t = 1-alpha. Expanding, main+corr = X@W + noise where the e'
     terms cancel exactly (e' is known offline) and the residual is
     ~[2t^2 + (delta*alpha/t)^2]*delta^2, minimized at t ~ sqrt(delta):
     t=1/8 measures 0.035x the plain-class residual err^2 — 7x better
     than the classic half-Wh mixed pass — for the same single extra
     pass. This obsoletes the 2-pass exact correction everywhere.

Partial-k coverage (kcov, in DoubleRow k-pairs of 256) interpolates cost
vs err^2 linearly: covered k-tiles use Wt + A@B, uncovered use plain Wh.

Weight columns are interleaved as c = oblk*512 + gate*128 + (o % 128)
with gate order [i,f,o,a]. Per-mat slab layout: [main(512) | B cols of
full-k S ranges]; the partial-k f-gate B block ships as a tiny separate
tensor (bf0) so the slab stays rectangular without dead bytes.
"""

import sys

for _p in ("/opt/trn_rl_repo",):
    if _p not in sys.path:
        sys.path.insert(0, _p)

import numpy as np
import ml_dtypes

import concourse.bass as bass
import concourse.mybir as mybir
from concourse.bass_utils import run_bass_kernel_spmd
from concourse.tile import TileContext

F32 = mybir.dt.float32
BF16 = mybir.dt.bfloat16
F8 = mybir.dt.float8e4
AFT = mybir.ActivationFunctionType
DR = mybir.MatmulPerfMode.DoubleRow

B = 4096
IN = 1024
H = 1024
G = 4
NCORES = 8
BL = B // NCORES          # 512 batch rows per core
MT = BL // 128            # 4 m-tiles per core
K = 2 * IN                # 2048 contraction dim (x|h concat)
KT = K // 128             # 16 k-tiles
OB = H // 128             # 8 o-blocks
NW = G * 128              # 512 matmul N (all gates for one o-block)
SX = 4.0
SW = 1024.0
SP = SX * SW
TS = 0.125                # M* split t; alpha = 1 - t
AL = 1.0 - TS

# (c0, c1, class, kcov) over the 512 gate columns, order [i, f, o, a].
# Offline-LP solution: sim h_t rel 1.942e-2 / c_t 1.788e-2 vs the 2e-2
# gate (deterministic inputs; hw-sim gap measured ~1e-7 in err^2).
CFG = [
    [(0, 128, "P", 0), (128, 256, "S", 1), (256, 512, "S", 8)],  # P1
    [(0, 256, "P", 0), (256, 512, "S", 8)],                      # P2
    [(0, 512, "S", 8)],                                          # P3
]


def _mat_passes(mat):
    """[(c0, c1, corr_off, kcov)] for slab-resident S ranges (kcov==8),
    plus slab width. Partial-k S ranges ride the bf0 side tensor."""
    off = NW
    passes = []
    for (c0, c1, cls, kcov) in CFG[mat]:
        if cls == "S" and kcov == 8:
            passes.append((c0, c1, off, kcov))
            off += c1 - c0
    return passes, off


MAT_PASSES = [_mat_passes(m) for m in range(3)]
WIDTHS = [MAT_PASSES[m][1] for m in range(3)]  # 768, 768, 1024
# the one partial-k S range (mat0 f-gate, kcov=1): B block [OB,128,2,128]
BF0_MAT, BF0_C0, BF0_C1, BF0_KCOV = 0, 128, 256, 1


def _split_multiwait_json(raw: bytes) -> bytes:
    """The walrus build in this container accepts at most one sem wait
    per instruction; Tile's scheduler packs several. Split the extras
    into preceding wait-only EventSemaphore instructions on the same
    engine (same semantics: the sequencer blocks on each in order)."""
    import orjson

    m = orjson.loads(raw)
    ctr = 0
    for fn in m["functions"]:
        for bb in fn["blocks"]:
            out = []
            for ins in bb["instructions"]:
                si = ins.get("sync_info")
                waits = si.get("on_wait") if si else None
                if waits and len(waits) > 1:
                    for w in waits[:-1]:
                        ctr += 1
                        nop = {
                            "engine": ins["engine"],
                            "ins": [],
                            "outs": [],
                            "name": f"{ins['name']}_sw{ctr}",
                            "opcode": "EventSemaphore",
                            "sync_info": {"on_update": [], "on_wait": [w]},
                        }
                        if "debug" in ins:
                            nop["debug"] = ins["debug"]
                        out.append(nop)
                    si["on_wait"] = [waits[-1]]
                out.append(ins)
            bb["instructions"] = out
    return orjson.dumps(m)


def _build_program():
    nc = bass.Bass()

    xh = [nc.dram_tensor(f"x{m}h", [KT, 128, BL], F8, kind="ExternalInput")
          for m in range(3)]
    xa = [nc.dram_tensor(f"x{m}a", [KT, 128, BL], F8, kind="ExternalInput")
          for m in range(3)]
    cx = nc.dram_tensor("cx", [BL, 2 * H], BF16, kind="ExternalInput")
    wt = [nc.dram_tensor(f"w{m}", [OB, 128, KT, WIDTHS[m]], F8,
                         kind="ExternalInput") for m in range(3)]
    bf0 = nc.dram_tensor("bf0", [OB, 128, 2 * BF0_KCOV, BF0_C1 - BF0_C0], F8,
                         kind="ExternalInput")
    bbc = nc.dram_tensor("bbc", [2, 128, G * H], BF16, kind="ExternalInput")
    h_out = nc.dram_tensor("h_out", [BL, 2 * H], BF16, kind="ExternalOutput")
    c_out = nc.dram_tensor("c_out", [BL, 2 * H], BF16, kind="ExternalOutput")

    with TileContext(nc) as tc:
        with (
            tc.tile_pool(name="const", bufs=2) as constp,
            tc.tile_pool(name="cres", bufs=2) as cresp,
            tc.tile_pool(name="xt", bufs=1) as xtp,
            tc.tile_pool(name="w0p", bufs=2) as w0p,
            tc.tile_pool(name="w1p", bufs=2) as w1p,
            tc.tile_pool(name="w2p", bufs=2) as w2p,
            tc.tile_pool(name="ep", bufs=2) as epp,
            tc.tile_pool(name="prod", bufs=2) as prodp,
            tc.tile_pool(name="ps_mm", bufs=6, space="PSUM") as psmm,
        ):
            wpools = [w0p, w1p, w2p]
            obres = {}
            bfts = {}

            def load_bf0(ob, eng):
                bft = constp.tile([128, 2 * BF0_KCOV, BF0_C1 - BF0_C0], F8,
                                  tag="bf0", name="bf0")
                eng.dma_start(out=bft[:], in_=bf0[ob])
                bfts[ob] = bft

            def load_ob_resources(ob):
                brt = constp.tile([128, NW], BF16, tag="bias_r", name="bias_r")
                bit = constp.tile([128, NW], BF16, tag="bias_i", name="bias_i")
                obw = slice(ob * NW, (ob + 1) * NW)
                nc.scalar.dma_start(out=brt[:], in_=bbc[0][:, obw])
                nc.scalar.dma_start(out=bit[:], in_=bbc[1][:, obw])
                if ob not in bfts:
                    load_bf0(ob, nc.scalar)
                cts = []
                for m in range(MT):
                    t = cresp.tile([128, 2, 128], BF16, tag=f"c_m{m}",
                                   name=f"c_m{m}")
                    src = cx[m * 128 : (m + 1) * 128, :].rearrange(
                        "r (two h) -> r two h", two=2
                    )[:, :, ob * 128 : (ob + 1) * 128]
                    nc.scalar.dma_start(out=t[:], in_=src)
                    cts.append(t)
                obres[ob] = (brt, bit, cts)

            xht = [xtp.tile([128, KT, BL], F8, tag=f"xh{m}", name=f"xh{m}")
                   for m in range(3)]
            att = [xtp.tile([128, KT, BL], F8, tag=f"xa{m}", name=f"xa{m}")
                   for m in range(3)]

            def emit_xload(src, dst, eng, quarters=4, peel=False, kt=KT):
                kq = kt // quarters
                for q in range(quarters):
                    if peel and q == 0:
                        eng.dma_start(
                            out=dst[:, 0:2, :],
                            in_=src[0:2].rearrange("kt p b -> p kt b"))
                        eng.dma_start(
                            out=dst[:, 2:4, :],
                            in_=src[2:4].rearrange("kt p b -> p kt b"))
                        continue
                    eng.dma_start(
                        out=dst[:, q * kq : (q + 1) * kq, :],
                        in_=src[q * kq : (q + 1) * kq].rearrange(
                            "kt p b -> p kt b"))

            def load_slab(ob, mat, chunks=1, eng2=None):
                """eng2: second DMA ring carrying the back half (catch-up
                relief for the SP weight stream)."""
                w = wpools[mat].tile([128, KT, WIDTHS[mat]], F8, tag="wslab",
                                     name=f"w{mat}slab")
                wsrc = wt[mat][ob].rearrange("p kt c -> p (kt c)")
                wdst = w.rearrange("p kt c -> p (kt c)")
                tot = KT * WIDTHS[mat]
                if eng2 is not None:
                    half = tot // 2
                    nc.sync.dma_start(out=wdst[:, :half], in_=wsrc[:, :half])
                    eng2.dma_start(out=wdst[:, half:], in_=wsrc[:, half:])
                    return w
                cs = tot // chunks
                for q in range(chunks):
                    nc.sync.dma_start(out=wdst[:, q * cs : (q + 1) * cs],
                                      in_=wsrc[:, q * cs : (q + 1) * cs])
                return w

            p1ps = [None] * MT   # P1 PSUM handles per m
            pa = [None] * MT     # phase-A products per m

            def emit_passes(ps, rows, mat, w, ob, which, start, stop,
                            c0lim=0, c1lim=NW):
                """which: subset of ('m','a','b'). b = the bf0 partial-k
                pass (mat0 f-gate). start/stop on first/last instruction."""
                plist = []
                if "m" in which:
                    plist.append(("m", c0lim, c1lim, 0, KT // 2))
                if "a" in which:
                    for (c0, c1, off, kcov) in MAT_PASSES[mat][0]:
                        d0, d1 = max(c0, c0lim), min(c1, c1lim)
                        if d0 < d1:
                            plist.append(("a", d0, d1, off + d0 - c0, kcov))
                if "b" in which and mat == BF0_MAT:
                    d0, d1 = max(BF0_C0, c0lim), min(BF0_C1, c1lim)
                    if d0 < d1:
                        plist.append(("b", d0, d1, d0 - BF0_C0, BF0_KCOV))
                for ci, (side, d0, d1, off, kcov) in enumerate(plist):
                    for kp in range(kcov):
                        k = 2 * kp
                        if side == "m":
                            lhsT = xht[mat][:, k : k + 2, rows]
                            rhs = w[:, k : k + 2, d0:d1]
                        elif side == "a":
                            lhsT = att[mat][:, k : k + 2, rows]
                            rhs = w[:, k : k + 2, off : off + d1 - d0]
                        else:
                            lhsT = att[mat][:, k : k + 2, rows]
                            rhs = bfts[ob][:, k : k + 2, off : off + d1 - d0]
                        nc.tensor.matmul(
                            ps[:, d0:d1],
                            lhsT=lhsT,
                            rhs=rhs,
                            start=(start and ci == 0 and kp == 0),
                            stop=(stop and ci == len(plist) - 1
                                  and kp == kcov - 1),
                            perf_mode=DR,
                        )

            def phase_a(ob, m, ps2):
                # everything that only needs P1/P2 (not P3); overlaps the
                # P3 matmuls. Gate cols: i=[0:128] f=[128:256] o=[256:384]
                # a=[384:512]. PSUM holds SP*z. Tensor ops may read only
                # ONE PSUM operand, so P1 gets an SBUF copy (DVE) first.
                brt, bit, cts = obres[ob]
                p1 = p1ps[m]
                zr = epp.tile([128, NW], F32, tag="zra", name="zra")
                nc.vector.tensor_sub(zr[:], p1[:], ps2[:])
                nc.gpsimd.tensor_add(zr[:], zr[:], brt[:])
                gr = epp.tile([128, NW], F32, tag=f"gr_{m}",
                              name=f"gr_{m}", bufs=1)
                nc.scalar.activation(gr[:, 0:384], zr[:, 0:384],
                                     AFT.Sigmoid, scale=1.0 / SP)
                nc.scalar.activation(gr[:, 384:512], zr[:, 384:512],
                                     AFT.Tanh, scale=1.0 / SP)
                q = epp.tile([128, NW], F32, tag=f"q_{m}",
                             name=f"q_{m}", bufs=1)
                nc.vector.tensor_add(q[:], p1[:], ps2[:])
                nc.gpsimd.tensor_sub(q[:], q[:], bit[:])
                cr = cts[m][:, 0, :]
                ci = cts[m][:, 1, :]
                ir_ = gr[:, 0:128]
                fr = gr[:, 128:256]
                ar = gr[:, 384:512]
                u1 = prodp.tile([128, 128], F32, tag=f"u1_{m}",
                                name=f"u1_{m}", bufs=1)
                u4 = prodp.tile([128, 128], F32, tag=f"u4_{m}",
                                name=f"u4_{m}", bufs=1)
                v1 = prodp.tile([128, 128], F32, tag=f"v1_{m}",
                                name=f"v1_{m}", bufs=1)
                nc.gpsimd.tensor_mul(u1[:], cr, fr)
                nc.gpsimd.tensor_mul(u4[:], ci, fr)
                nc.gpsimd.tensor_mul(v1[:], ar, ir_)
                pa[m] = (gr, q, u1, u4, v1)

            def phase_b(ob, m, rows, zi_parts, last_ob=False, tail=False):
                """zi_parts: [(ps, c0, c1, func)] in ACT-queue order. For
                tail=True the c-product chain rides Pool exclusively (DVE
                keeps only the PSUM subs) and is emitted after the o-gate
                part so the o sigmoid is not queued behind the ct tanhs."""
                gr, q, u1, u4, v1 = pa[m]
                zi = epp.tile([128, NW], F32, tag="zi", name="zi")
                gi = epp.tile([128, NW], F32, tag="gi", name="gi")
                D, P = nc.vector, nc.gpsimd
                # engine map: c-path ops
                if tail:
                    e_u3 = e_v3 = e_cfi = P
                else:
                    e_u3 = e_v3 = e_cfi = D
                emitted = []
                cpath = {}

                def c_products():
                    cr = obres[ob][2][m][:, 0, :]
                    ci = obres[ob][2][m][:, 1, :]
                    ii_ = gi[:, 0:128]
                    fi = gi[:, 128:256]
                    ai = gi[:, 384:512]
                    ir_ = gr[:, 0:128]
                    ar = gr[:, 384:512]
                    u2 = prodp.tile([128, 128], F32, tag="u2", name="u2", bufs=1)
                    u3 = prodp.tile([128, 128], F32, tag="u3", name="u3", bufs=1)
                    v2 = prodp.tile([128, 128], F32, tag="v2", name="v2", bufs=1)
                    v3 = prodp.tile([128, 128], F32, tag="v3", name="v3", bufs=1)
                    v4 = prodp.tile([128, 128], F32, tag="v4", name="v4", bufs=1)
                    P.tensor_mul(u2[:], ci, fi)
                    e_u3.tensor_mul(u3[:], cr, fi)
                    P.tensor_mul(v2[:], ai, ii_)
                    e_v3.tensor_mul(v3[:], ar, ii_)
                    P.tensor_mul(v4[:], ai, ir_)
                    cfr = prodp.tile([128, 128], F32, tag="cfr", name="cfr", bufs=1)
                    cfi = prodp.tile([128, 128], F32, tag="cfi", name="cfi", bufs=1)
                    air = prodp.tile([128, 128], F32, tag="air", name="air", bufs=1)
                    aii = prodp.tile([128, 128], F32, tag="aii", name="aii", bufs=1)
                    P.tensor_sub(cfr[:], u1[:], u2[:])
                    e_cfi.tensor_add(cfi[:], u3[:], u4[:])
                    P.tensor_sub(air[:], v1[:], v2[:])
                    P.tensor_add(aii[:], v3[:], v4[:])
                    ct2 = prodp.tile([128, 2, 128], BF16, tag="ct2", name="ct2")
                    ctr = ct2[:, 0, :]
                    cti = ct2[:, 1, :]
                    P.tensor_add(ctr, cfr[:], air[:])
                    P.tensor_add(cti, cfi[:], aii[:])
                    # one 256-wide tanh covers both halves of ct
                    tt = prodp.tile([128, 2, 128], F32, tag="tt", name="tt")
                    nc.scalar.activation(tt[:], ct2[:], AFT.Tanh)
                    cpath.update(ct2=ct2, tr=tt[:, 0, :], ti=tt[:, 1, :])

                for (psx, c0, c1, func) in zi_parts:
                    nc.vector.tensor_sub(zi[:, c0:c1], psx[:, c0:c1],
                                         q[:, c0:c1])
                    nc.scalar.activation(gi[:, c0:c1], zi[:, c0:c1],
                                         func, scale=1.0 / SP)
                    emitted.append((c0, c1))
                    have_a = any(cc1 >= 512 for (cc0, cc1) in emitted)
                    have_if = any(cc0 == 0 for (cc0, cc1) in emitted)
                    if have_a and have_if and not cpath and not tail:
                        c_products()
                if not cpath:
                    c_products()

                ct2, tr, ti = cpath["ct2"], cpath["tr"], cpath["ti"]
                orr = gr[:, 256:384]
                oi = gi[:, 256:384]
                ht2 = prodp.tile([128, 2, 128], BF16, tag="ht2", name="ht2")
                htr = ht2[:, 0, :]
                hti = ht2[:, 1, :]
                w1 = prodp.tile([128, 128], F32, tag="w1", name="w1")
                w2 = prodp.tile([128, 128], F32, tag="w2", name="w2")
                w3 = prodp.tile([128, 128], F32, tag="w3", name="w3")
                w4 = prodp.tile([128, 128], F32, tag="w4", name="w4")
                if tail:
                    P.tensor_mul(w1[:], orr, tr[:])
                    P.tensor_mul(w2[:], oi, ti[:])
                    P.tensor_sub(htr, w1[:], w2[:])
                    D.tensor_mul(w3[:], orr, ti[:])
                    D.tensor_mul(w4[:], oi, tr[:])
                    D.tensor_add(hti, w3[:], w4[:])
                else:
                    D.tensor_mul(w1[:], orr, tr[:])
                    P.tensor_mul(w2[:], oi, ti[:])
                    D.tensor_sub(htr, w1[:], w2[:])
                    P.tensor_mul(w3[:], orr, ti[:])
                    D.tensor_mul(w4[:], oi, tr[:])
                    P.tensor_add(hti, w3[:], w4[:])
                if last_ob:
                    oe1, oe2 = nc.sync, nc.sync
                else:
                    oe1, oe2 = nc.scalar, nc.gpsimd
                hdst = h_out[rows, :].rearrange(
                    "r (two h) -> r two h", two=2
                )[:, :, ob * 128 : (ob + 1) * 128]
                cdst = c_out[rows, :].rearrange(
                    "r (two h) -> r two h", two=2
                )[:, :, ob * 128 : (ob + 1) * 128]
                oe1.dma_start(out=hdst, in_=ht2[:])
                oe2.dma_start(out=cdst, in_=ct2[:])

            def finish_mat(ob, mat, m, ps, last_ob=False):
                if mat == 0:
                    p1c = epp.tile([128, NW], F32, tag=f"p1_{m}",
                                   name=f"p1_{m}", bufs=1)
                    nc.vector.tensor_copy(p1c[:], ps[:])
                    p1ps[m] = p1c
                elif mat == 1:
                    phase_a(ob, m, ps)
                else:
                    phase_b(ob, m, slice(m * 128, (m + 1) * 128),
                            [(ps, 384, 512, AFT.Tanh),
                             (ps, 0, 384, AFT.Sigmoid)],
                            last_ob=last_ob)

            # ---------------- emission ----------------
            emit_xload(xh[0], xht[0], nc.scalar, peel=True)
            emit_xload(xa[0], att[0], nc.gpsimd)
            load_bf0(0, nc.gpsimd)

            # ob0 mat0: mains deferred from corr passes (the A/B operands
            # land after the first mains start). Only mat0 defers — with
            # P1 banks held to phase A, deferring mat1 would deadlock the
            # in-order PE queue on PSUM-buffer reuse.
            w00 = load_slab(0, 0, chunks=8)
            ob0_ps = []
            for m in range(MT):
                ps = psmm.tile([128, NW], F32, tag="mm", name="mm")
                emit_passes(ps, slice(m * 128, (m + 1) * 128), 0, w00, 0,
                            ("m",), start=True, stop=False)
                ob0_ps.append(ps)
            for m in range(MT):
                emit_passes(ob0_ps[m], slice(m * 128, (m + 1) * 128), 0,
                            w00, 0, ("a", "b"), start=False, stop=True)
                finish_mat(0, 0, m, ob0_ps[m])
            # xh1 halves ride both rings so mat1's mains unblock sooner;
            # w1(ob0)'s back half rides the ACT ring (ahead of xa1/bias in
            # that queue) so the SP stream reaches w2(ob0) 2.4us earlier
            emit_xload(xh[1][:8], xht[1][:, :8, :], nc.scalar, quarters=2,
                       kt=8)
            emit_xload(xh[1][8:], xht[1][:, 8:, :], nc.gpsimd, quarters=2,
                       kt=8)
            w1_0 = load_slab(0, 1, eng2=nc.scalar)
            emit_xload(xa[1], att[1], nc.scalar)
            emit_xload(xh[2], xht[2], nc.gpsimd)
            emit_xload(xa[2], att[2], nc.gpsimd)
            load_ob_resources(0)
            for mat, w in ((1, w1_0), (2, load_slab(0, 2, chunks=4))):
                for m in range(MT):
                    ps = psmm.tile([128, NW], F32, tag="mm", name="mm")
                    emit_passes(ps, slice(m * 128, (m + 1) * 128), mat, w, 0,
                                ("m", "a", "b"), start=True, stop=True)
                    finish_mat(0, mat, m, ps)

            for ob in range(1, OB):
                last = ob == OB - 1
                for mat in range(3):
                    # the SP weight stream runs behind after ob0's bursty
                    # start; the big mat2 slabs of ob1-2 ride two rings
                    eng2 = nc.gpsimd if (mat == 2 and ob <= 2) else None
                    w = load_slab(ob, mat, eng2=eng2)
                    if mat == 0:
                        load_ob_resources(ob)
                    for m in range(MT):
                        rows = slice(m * 128, (m + 1) * 128)
                        if last and mat == 2 and m >= 2:
                            # tail: three accumulation groups; the h-only
                            # o-gate closes last so the c-path overlaps
                            # the o matmuls
                            psIF = psmm.tile([128, NW], F32, tag="mm", name="mm")
                            emit_passes(psIF, rows, mat, w, ob, ("m", "a", "b"),
                                        True, True, c0lim=0, c1lim=256)
                            psA = psmm.tile([128, NW], F32, tag="mm", name="mm")
                            emit_passes(psA, rows, mat, w, ob, ("m", "a", "b"),
                                        True, True, c0lim=384, c1lim=512)
                            psO = psmm.tile([128, NW], F32, tag="mm", name="mm")
                            emit_passes(psO, rows, mat, w, ob, ("m", "a", "b"),
                                        True, True, c0lim=256, c1lim=384)
                            phase_b(ob, m, rows,
                                    [(psIF, 0, 256, AFT.Sigmoid),
                                     (psA, 384, 512, AFT.Tanh),
                                     (psO, 256, 384, AFT.Sigmoid)],
                                    last_ob=True, tail=True)
                        else:
                            ps = psmm.tile([128, NW], F32, tag="mm", name="mm")
                            emit_passes(ps, rows, mat, w, ob, ("m", "a", "b"),
                                        True, True)
                            finish_mat(ob, mat, m, ps, last_ob=last)
    return nc


_NC_CACHE = None


def _get_program():
    global _NC_CACHE
    if _NC_CACHE is None:
        nc = _build_program()
        fixed = _split_multiwait_json(nc.to_json_bytes())
        nc.to_json_bytes = lambda: fixed
        _NC_CACHE = nc
    return _NC_CACHE


F8NP = ml_dtypes.float8_e4m3


def _split8(a):
    a = a * SX
    ah = a.astype(F8NP)
    ahf = ah.astype(np.float32)
    al = (a - ahf).astype(F8NP)
    aa = (ahf + al.astype(np.float32) / TS).astype(F8NP)
    return ah, aa


def _pack_weights(Uw_r, Uw_i, Ub_r, Ub_i, Ww_r, Ww_i, Wb_r, Wb_i):
    GORD = [1, 0, 3, 2]  # column blocks [i, f, o, a]

    def interleave_cols(Wg):  # [2048, G, H] -> [2048, GH]
        return (
            Wg.reshape(K, G, OB, 128)[:, GORD]
            .transpose(0, 2, 1, 3)
            .reshape(K, G * H)
        )

    Wr = np.concatenate(
        [np.transpose(Uw_r, (2, 0, 1)), np.transpose(Ww_r, (2, 0, 1))], axis=0
    )
    Wi = np.concatenate(
        [np.transpose(Uw_i, (2, 0, 1)), np.transpose(Ww_i, (2, 0, 1))], axis=0
    )
    W1 = interleave_cols(Wr) * SW
    W2 = interleave_cols(Wi) * SW
    W3 = W1 + W2
    Wall = [W1, W2, W3]

    def slabify(Wm):  # [K, G*H] -> [OB, 128, KT, NW]
        return Wm.reshape(KT, 128, OB, NW).transpose(2, 1, 0, 3)

    slabs = []
    bf0_pack = None
    for m in range(3):
        W = Wall[m]
        Wh8 = W.astype(F8NP)
        Wt8 = (AL * W).astype(F8NP)
        ep = Wt8.astype(np.float32) - AL * W
        B8 = (TS * W - ep).astype(F8NP)
        # main region: per range, per k: Wt for covered k, Wh beyond
        main = Wh8.astype(np.float32).reshape(K, OB, NW)
        for (c0, c1, cls, kcov) in CFG[m]:
            if cls == "S":
                main[: kcov * 256, :, c0:c1] = (
                    Wt8.astype(np.float32).reshape(K, OB, NW)[: kcov * 256, :, c0:c1]
                )
        parts = [slabify(main.reshape(K, G * H).astype(F8NP))]
        for (c0, c1, off, kcov) in MAT_PASSES[m][0]:
            parts.append(slabify(B8)[:, :, :, c0:c1])
        slabs.append(np.ascontiguousarray(np.concatenate(parts, axis=-1)))
        if m == BF0_MAT:
            # [OB, 128, 2*kcov, cols]
            bf0_pack = np.ascontiguousarray(
                slabify(B8)[:, :, : 2 * BF0_KCOV, BF0_C0:BF0_C1]
            )

    def interleave_bias(b):  # [G, H] -> [GH] interleaved, pre-scaled
        return b.reshape(G, OB, 128)[GORD].transpose(1, 0, 2).reshape(G * H)

    br = interleave_bias((Ub_r + Wb_r) * SP)
    bi = interleave_bias((Ub_i + Wb_i) * SP)
    bbc = np.ascontiguousarray(np.broadcast_to(
        np.stack([br, bi])[:, None, :], (2, 128, G * H)
    ).astype(ml_dtypes.bfloat16))
    return slabs, bf0_pack, bbc


def kernel(input, h_x, c_x, Uw_r, Uw_i, Ub_r, Ub_i, Ww_r, Ww_i, Wb_r, Wb_i,
           _trace=False):
    input = np.asarray(input, dtype=np.float32)
    h_x = np.asarray(h_x, dtype=np.float32)
    c_x = np.asarray(c_x, dtype=np.float32)
    slabs, bf0_pack, bpk = _pack_weights(
        np.asarray(Uw_r, np.float32), np.asarray(Uw_i, np.float32),
        np.asarray(Ub_r, np.float32), np.asarray(Ub_i, np.float32),
        np.asarray(Ww_r, np.float32), np.asarray(Ww_i, np.float32),
        np.asarray(Wb_r, np.float32), np.asarray(Wb_i, np.float32),
    )

    X1 = np.concatenate([input[:, :IN], h_x[:, :H]], axis=1)
    X2 = np.concatenate([input[:, IN:], h_x[:, H:]], axis=1)
    X3 = X1 + X2
    xparts = [_split8(X) for X in (X1, X2, X3)]

    in_maps = []
    for c in range(NCORES):
        rows = slice(c * BL, (c + 1) * BL)
        im = {
            "cx": np.ascontiguousarray(c_x[rows].astype(ml_dtypes.bfloat16)),
            "bbc": bpk,
            "bf0": bf0_pack,
        }
        for m in range(3):
            im[f"w{m}"] = slabs[m]
            xhm, xam = xparts[m]
            im[f"x{m}h"] = np.ascontiguousarray(
                xhm[rows].T.reshape(KT, 128, BL)
            )
            im[f"x{m}a"] = np.ascontiguousarray(
                xam[rows].T.reshape(KT, 128, BL)
            )
        in_maps.append(im)

    nc = _get_program()
    res = run_bass_kernel_spmd(
        nc, in_maps, core_ids=list(range(NCORES)), trace=_trace
    )
    h_t = np.concatenate(
        [res.results[i]["h_out"].astype(np.float32) for i in range(NCORES)],
        axis=0,
    )
    c_t = np.concatenate(
        [res.results[i]["c_out"].astype(np.float32) for i in range(NCORES)],
        axis=0,
    )
    if _trace:
        kernel._last_results = res
    return h_t, c_t
